# revision 21
# baseline (speedup 1.0000x reference)
"""Trainium2 Bass kernel for nn_Attention_17738214932808.

Computation (per batch b):
    mids   = q @ W.T                               [B, D]
    scores = tanh(k . mids + bias)                 [B, T]
    attn   = softmax-with-mask:  e = exp(scores - max) * m ; attn = e / sum(e)
Since tanh is bounded in (-1, 1), the max-subtraction is a mathematical no-op
for the final ratio (exp(s-c)/sum m exp(s-c) is invariant in c), so we compute
e = exp(scores) * m directly; fp32-rounding-level difference only.

Sharding: data-parallel over batch, 8 batches per NeuronCore x 8 cores.

Layout trick: each SBUF partition loads a CONTIGUOUS 16KB run of k (16 t-rows),
so k's DMA runs at ~HBM peak. The resulting score-column permutation is fixed
up by pre-permuting m and inverse-permuting the output on the host (pure input
marshalling; all FLOPs happen on-device).

Per-core engine split for the hot dot-product loop (t-subtile granularity):
  - DVE: fused multiply+reduce (tensor_tensor_reduce) for some subtiles,
         plain multiplies (tensor_tensor) for the rest
  - ACT: reduces the plain-multiply outputs via activation(Copy, accum_out=)
This balances DVE/ACT at ~half the elements each so both hide under the DMA.
"""

import os

import numpy as np

import concourse.bass as bass
import concourse.tile as tile
from concourse import bacc, mybir
from concourse.bass_utils import run_bass_kernel_spmd
from concourse.masks import make_identity

F32 = mybir.dt.float32
AF = mybir.ActivationFunctionType
ALU = mybir.AluOpType

B, T, D = 64, 4096, 256
NCORES = 8
BL = B // NCORES          # batches per core = 8
H = 2                     # halves of T per batch (macro tiles)
TT = 16                   # t-subtiles per macro  (T = H * 128 * TT)
P = 128

# Engine split per macro (16 subtiles): first N_TTR via fused DVE op
# (scalar_tensor_tensor w/ accum), the rest in chunks via DVE-mul + ACT-reduce.
N_TTR = 8
CHUNKS = (4, 4)           # partition of the remaining subtiles
assert N_TTR + sum(CHUNKS) == TT

LAST_RESULTS = None       # BassKernelResults of the most recent run (for test.py)


def _broadcast_row(ap, nparts):
    """[1, N] AP -> [nparts, N] AP with partition step 0."""
    try:
        return ap.to_broadcast([nparts] + list(ap.shape[1:]))
    except Exception:
        return bass.AP(
            tensor=ap.tensor,
            offset=ap.offset,
            ap=[[0, nparts]] + [list(d) for d in ap.ap[1:]],
        )


def _build_kernel(ctx, tc, outs, ins):
    nc = tc.nc
    q, k, mp, W, bias = ins["q"], ins["k"], ins["mp"], ins["W"], ins["bias"]
    out = outs["out"]

    consts = ctx.enter_context(tc.tile_pool(name="consts", bufs=1))
    setup = ctx.enter_context(tc.tile_pool(name="setup", bufs=2))
    kpool = ctx.enter_context(tc.tile_pool(name="kpool", bufs=6))
    scpool = ctx.enter_context(tc.tile_pool(name="scores", bufs=3))
    scratch = ctx.enter_context(tc.tile_pool(name="scratch", bufs=3))
    epool = ctx.enter_context(tc.tile_pool(name="epil", bufs=2))
    ps_misc = ctx.enter_context(tc.tile_pool(name="ps_misc", bufs=4, space="PSUM"))
    ps_e = ctx.enter_context(tc.tile_pool(name="ps_e", bufs=2, space="PSUM"))

    # ---------------- Phase 0: constants + mids = q @ W.T ----------------
    # W/q DMAs go first on the sync (HWDGE) queue, ahead of the k streams;
    # everything mids-related is latency-critical for the first dot products.
    w_sb = setup.tile([P, 2, D], F32, tag="w")
    nc.sync.dma_start(out=w_sb[:], in_=W.ap().rearrange("(dc p) e -> p dc e", p=P))
    q_sb = setup.tile([BL, D], F32, tag="q")
    nc.sync.dma_start(out=q_sb[:], in_=q.ap())

    ident = consts.tile([P, P], F32)
    make_identity(nc, ident)
    ones1 = consts.tile([1, P], F32)
    nc.vector.memset(ones1[:], 1.0)

    bias_col = consts.tile([P, 1], F32)
    nc.gpsimd.dma_start(out=bias_col[:], in_=_broadcast_row(bias.ap(), P))

    # ones block-diagonal [64, 2]: blk[p, g] = 1 iff p//32 == g
    blk = consts.tile([64, 2], F32)
    nc.gpsimd.memset(blk[:], 1.0)
    nc.gpsimd.affine_select(   # keep where p - 32g >= 0
        out=blk[:], in_=blk[:], compare_op=ALU.is_ge, fill=0.0,
        base=0, pattern=[[-32, 2]], channel_multiplier=1,
    )
    nc.gpsimd.affine_select(   # keep where 31 - p + 32g >= 0  (i.e. p - 32g <= 31)
        out=blk[:], in_=blk[:], compare_op=ALU.is_ge, fill=0.0,
        base=31, pattern=[[32, 2]], channel_multiplier=-1,
    )
    # selector [2, 64]: sel[g, x] = 1 iff x//32 == g
    sel = consts.tile([2, 64], F32)
    nc.gpsimd.memset(sel[:], 0.0)
    nc.gpsimd.affine_select(   # iota = g - (x//32); equal -> fill... keep where != 0
        out=sel.rearrange("p (g x) -> p g x", g=2),
        in_=sel.rearrange("p (g x) -> p g x", g=2),
        compare_op=ALU.not_equal, fill=1.0,
        base=0, pattern=[[-1, 2], [0, 32]], channel_multiplier=1,
    )

    # W^T chunks: wt[p=e_local, ec, dc, d_local]
    wt = setup.tile([P, 2, 2, P], F32, tag="wt")
    for dc in range(2):
        for ec in range(2):
            pst = ps_misc.tile([P, P], F32, tag="mix")
            nc.tensor.transpose(pst[:], w_sb[:, dc, ec * P:(ec + 1) * P], ident[:])
            nc.vector.tensor_copy(wt[:, ec, dc, :], pst[:])
    # q^T chunks: qt[p=e_local, ec, b]
    qt = setup.tile([P, 2, BL], F32, tag="qt")
    for ec in range(2):
        pst = ps_misc.tile([P, BL], F32, tag="mix")
        nc.tensor.transpose(pst[:], q_sb[:, ec * P:(ec + 1) * P], ident[0:BL, 0:BL])
        nc.vector.tensor_copy(qt[:, ec, :], pst[:])
    # midsT[d, b] = sum_e W[d, e] qT[e, b]  (accumulate over 2 e-chunks)
    midsT = setup.tile([P, 2, BL], F32, tag="midsT")
    for dc in range(2):
        psm = ps_misc.tile([P, BL], F32, tag="mix")
        for ec in range(2):
            nc.tensor.matmul(
                psm[:], lhsT=wt[:, ec, dc, :], rhs=qt[:, ec, :],
                start=(ec == 0), stop=(ec == 1),
            )
        nc.vector.tensor_copy(midsT[:, dc, :], psm[:])
    # mids rows [b, d]
    mids = setup.tile([BL, D], F32, tag="mids")
    for dc in range(2):
        psr = ps_misc.tile([BL, P], F32, tag="mix")
        nc.tensor.transpose(psr[:], midsT[:, dc, :], ident[:])
        nc.vector.tensor_copy(mids[:, dc * P:(dc + 1) * P], psr[:])
    # fold the 8 mids rows onto partition 0 (engines need 32-aligned partition
    # bases; partition 0 lets the ones-matmul below read any row as rhs)
    mids_flat = setup.tile([1, BL, D], F32, tag="mids_flat")
    nc.gpsimd.dma_start(out=mids_flat[:], in_=mids[:])

    # ---------------- Phase 1: main loop + epilogue per batch-pair ----------------
    ps_mb = ctx.enter_context(tc.tile_pool(name="ps_mb", bufs=2, space="PSUM"))
    for g in range(BL // 2):                 # 4 pairs
        scores = scpool.tile([P, 64], F32)   # col = b_local*32 + h*16 + tt
        # broadcast this pair's mids rows to all 128 partitions via a K=1
        # ones-matmul (PSUM result is read directly as the STT/TT in1)
        mb_ps = ps_mb.tile([P, 2, D], F32)
        for b_local in range(2):
            b = g * 2 + b_local
            nc.tensor.matmul(
                mb_ps[:, b_local, :], lhsT=ones1[:],
                rhs=mids_flat[:, b, :], start=True, stop=True,
            )
        for b_local in range(2):
            b = g * 2 + b_local
            mb = mb_ps[:, b_local, :]
            for h in range(2):
                kt = kpool.tile([P, TT, D], F32)
                nc.sync.dma_start(
                    out=kt[:],
                    in_=k.ap()[b, h * 2048:(h + 1) * 2048, :].rearrange(
                        "(p tt) d -> p tt d", p=P
                    ),
                )
                c0 = b_local * 32 + h * 16
                for tt in range(N_TTR):
                    sc = scratch.tile([P, D], F32, tag="ttr")
                    nc.vector.scalar_tensor_tensor(
                        out=sc[:], in0=kt[:, tt, :], scalar=0.0, in1=mb,
                        op0=ALU.bypass, op1=ALU.mult,
                        accum_out=scores[:, c0 + tt:c0 + tt + 1],
                    )
                tt0 = N_TTR
                for clen in CHUNKS:
                    tmp = scratch.tile([P, max(CHUNKS), D], F32, tag="mulchunk")
                    nc.vector.tensor_tensor(
                        out=tmp[:, 0:clen, :],
                        in0=kt[:, tt0:tt0 + clen, :],
                        in1=mb.unsqueeze(1).broadcast_to([P, clen, D]),
                        op=ALU.mult,
                    )
                    for i in range(clen):
                        asc = scratch.tile([P, D], F32, tag="actred")
                        nc.scalar.activation(
                            out=asc[:], in_=tmp[:, i, :], func=AF.Copy,
                            accum_out=scores[:, c0 + tt0 + i:c0 + tt0 + i + 1],
                        )
                    tt0 += clen

        # ---- epilogue for this pair of batches ----
        th = epool.tile([P, 64], F32, tag="th")
        nc.scalar.activation(out=th[:], in_=scores[:], func=AF.Tanh,
                             bias=bias_col[:], scale=1.0)
        ex = epool.tile([P, 64], F32, tag="ex")
        nc.scalar.activation(out=ex[:], in_=th[:], func=AF.Exp)
        pse = ps_e.tile([64, P], F32)
        nc.tensor.transpose(pse[:], ex[:], ident[:])

        mt = epool.tile([64, P], F32, tag="mt")
        nc.gpsimd.dma_start(
            out=mt[:],
            in_=mp.ap()[g * 2:(g + 1) * 2].rearrange("b c p -> (b c) p"),
        )
        ee = epool.tile([64, P], F32, tag="ee")
        rs = epool.tile([64, 1], F32, tag="rs")
        nc.vector.scalar_tensor_tensor(
            out=ee[:], in0=pse[:], scalar=0.0, in1=mt[:],
            op0=ALU.bypass, op1=ALU.mult, accum_out=rs[:],
        )
        pss = ps_misc.tile([2, 1], F32, tag="mix")
        nc.tensor.matmul(pss[:], lhsT=blk[:], rhs=rs[:], start=True, stop=True)
        rc = epool.tile([2, 1], F32, tag="rc")
        nc.vector.reciprocal(rc[:], pss[:])
        psr2 = ps_misc.tile([64, 1], F32, tag="mix")
        nc.tensor.matmul(psr2[:], lhsT=sel[:], rhs=rc[:], start=True, stop=True)
        rcol = epool.tile([64, 1], F32, tag="rcol")
        nc.vector.tensor_copy(rcol[:], psr2[:])
        attn = epool.tile([64, P], F32, tag="attn")
        nc.scalar.activation(out=attn[:], in_=ee[:], func=AF.Copy, scale=rcol[:])
        nc.gpsimd.dma_start(
            out=out.ap()[g * 2:(g + 1) * 2].rearrange("b c p -> (b c) p"),
            in_=attn[:],
        )


def _install_ntff_hook_shim():
    """Provide antenv.axon_hooks via ctypes into libaxon_pjrt.so (the agent
    image's antenv stub lacks it), enabling NTFF capture under trace=True."""
    import sys
    import types
    import ctypes
    import contextlib

    if "antenv.axon_hooks" in sys.modules:
        return
    so = "/opt/axon/libaxon_pjrt.so"
    if not os.path.exists(so):
        return
    lib = ctypes.CDLL(so)
    if not hasattr(lib, "axon_start_nrt_profile"):
        return
    lib.axon_start_nrt_profile.argtypes = [
        ctypes.POINTER(ctypes.c_int64), ctypes.c_size_t,
    ]
    lib.axon_start_nrt_profile.restype = ctypes.c_int64
    lib.axon_stop_nrt_profile.argtypes = [ctypes.c_char_p]
    lib.axon_stop_nrt_profile.restype = ctypes.c_int64

    @contextlib.contextmanager
    def _hook(output_dir, device_ids):
        import jax

        jax.devices()
        if device_ids:
            ids = (ctypes.c_int64 * len(device_ids))(*device_ids)
            rc = lib.axon_start_nrt_profile(ids, len(device_ids))
        else:
            rc = lib.axon_start_nrt_profile(None, 0)
        if rc != 0:
            raise RuntimeError(f"axon_start_nrt_profile rc={rc}")
        try:
            yield
        finally:
            n = lib.axon_stop_nrt_profile(str(output_dir).encode())
            print(f"profile: {n} file(s) written to {output_dir}", file=sys.stderr)

    mod = types.ModuleType("antenv.axon_hooks")
    mod.get_axon_ntff_profile_hook = lambda: _hook
    mod.set_axon_ntff_profile_hook = lambda h: None
    import antenv

    sys.modules["antenv.axon_hooks"] = mod
    antenv.axon_hooks = mod


_CACHE = {}


def _get_nc():
    if "nc" not in _CACHE:
        from contextlib import ExitStack

        nc = bacc.Bacc("TRN2", debug=False)
        ins = {
            "q": nc.dram_tensor("q", [BL, D], F32, kind="ExternalInput"),
            "k": nc.dram_tensor("k", [BL, T, D], F32, kind="ExternalInput"),
            "mp": nc.dram_tensor("mp", [BL, 32, P], F32, kind="ExternalInput"),
            "W": nc.dram_tensor("W", [D, D], F32, kind="ExternalInput"),
            "bias": nc.dram_tensor("bias", [1, 1], F32, kind="ExternalInput"),
        }
        outs = {"out": nc.dram_tensor("out", [BL, 32, P], F32, kind="ExternalOutput")}
        with tile.TileContext(nc) as tc:
            with ExitStack() as ctx:
                _build_kernel(ctx, tc, outs, ins)
        nc.compile()
        _CACHE["nc"] = nc
    return _CACHE["nc"]


def kernel(q, k, m, W, bias):
    global LAST_RESULTS
    q = np.ascontiguousarray(q, dtype=np.float32)
    k = np.ascontiguousarray(k, dtype=np.float32)
    m = np.ascontiguousarray(m, dtype=np.float32)
    W = np.ascontiguousarray(W, dtype=np.float32)
    bias = np.ascontiguousarray(bias, dtype=np.float32).reshape(1, 1)

    # host-side input marshalling: permute m to the kernel's score layout
    # mp[b, h*16+tt, p] = m[b, h*2048 + p*16 + tt]
    mp = np.ascontiguousarray(
        m.reshape(B, H, P, TT).transpose(0, 1, 3, 2).reshape(B, H * TT, P)
    )

    trace = bool(int(os.environ.get("KERNEL_TRACE", "0")))
    if trace:
        _install_ntff_hook_shim()
    nc = _get_nc()
    in_maps = [
        {
            "q": q[i * BL:(i + 1) * BL],
            "k": k[i * BL:(i + 1) * BL],
            "mp": mp[i * BL:(i + 1) * BL],
            "W": W,
            "bias": bias,
        }
        for i in range(NCORES)
    ]
    res = run_bass_kernel_spmd(
        nc,
        in_maps,
        core_ids=list(range(NCORES)),
        trace=trace,
    )
    LAST_RESULTS = res

    full = np.concatenate([res.results[i]["out"] for i in range(NCORES)], axis=0)
    # inverse permutation back to natural [B, T]
    out = np.ascontiguousarray(
        full.reshape(B, H, TT, P).transpose(0, 1, 3, 2).reshape(B, T)
    )
    return out


# revision 22
# speedup vs baseline: 1.0344x; 1.0344x over previous
"""Trainium2 Bass kernel for nn_Attention_17738214932808.

Computation (per batch b):
    mids   = q @ W.T                               [B, D]
    scores = tanh(k . mids + bias)                 [B, T]
    attn   = softmax-with-mask:  e = exp(scores - max) * m ; attn = e / sum(e)
Since tanh is bounded in (-1, 1), the max-subtraction is a mathematical no-op
for the final ratio (exp(s-c)/sum m exp(s-c) is invariant in c), so we compute
e = exp(scores) * m directly; fp32-rounding-level difference only.

Sharding: data-parallel over batch, 8 batches per NeuronCore x 8 cores.

Layout trick: each SBUF partition loads a CONTIGUOUS 16KB run of k (16 t-rows),
so k's DMA runs at ~HBM peak. The resulting score-column permutation is fixed
up by pre-permuting m and inverse-permuting the output on the host (pure input
marshalling; all FLOPs happen on-device).

Per-core engine split for the hot dot-product loop (t-subtile granularity):
  - DVE: fused multiply+reduce (tensor_tensor_reduce) for some subtiles,
         plain multiplies (tensor_tensor) for the rest
  - ACT: reduces the plain-multiply outputs via activation(Copy, accum_out=)
This balances DVE/ACT at ~half the elements each so both hide under the DMA.
"""

import os

import numpy as np

import concourse.bass as bass
import concourse.tile as tile
from concourse import bacc, mybir
from concourse.bass_utils import run_bass_kernel_spmd
from concourse.masks import make_identity

F32 = mybir.dt.float32
AF = mybir.ActivationFunctionType
ALU = mybir.AluOpType

B, T, D = 64, 4096, 256
NCORES = 8
BL = B // NCORES          # batches per core = 8
H = 2                     # halves of T per batch (macro tiles)
TT = 16                   # t-subtiles per macro  (T = H * 128 * TT)
P = 128

# Engine split per macro (16 subtiles): first N_TTR via fused DVE op
# (scalar_tensor_tensor w/ accum), the rest in chunks via DVE-mul + ACT-reduce.
N_TTR = 8
CHUNKS = (4, 4)           # partition of the remaining subtiles
assert N_TTR + sum(CHUNKS) == TT

LAST_RESULTS = None       # BassKernelResults of the most recent run (for test.py)


def _broadcast_row(ap, nparts):
    """[1, N] AP -> [nparts, N] AP with partition step 0."""
    try:
        return ap.to_broadcast([nparts] + list(ap.shape[1:]))
    except Exception:
        return bass.AP(
            tensor=ap.tensor,
            offset=ap.offset,
            ap=[[0, nparts]] + [list(d) for d in ap.ap[1:]],
        )


def _build_kernel(ctx, tc, outs, ins):
    nc = tc.nc
    q, k, mp, W, bias = ins["q"], ins["k"], ins["mp"], ins["W"], ins["bias"]
    out = outs["out"]

    consts = ctx.enter_context(tc.tile_pool(name="consts", bufs=1))
    setup = ctx.enter_context(tc.tile_pool(name="setup", bufs=2))
    kpool = ctx.enter_context(tc.tile_pool(name="kpool", bufs=6))
    scpool = ctx.enter_context(tc.tile_pool(name="scores", bufs=3))
    scratch = ctx.enter_context(tc.tile_pool(name="scratch", bufs=3))
    epool = ctx.enter_context(tc.tile_pool(name="epil", bufs=2))
    ps_misc = ctx.enter_context(tc.tile_pool(name="ps_misc", bufs=4, space="PSUM"))
    ps_e = ctx.enter_context(tc.tile_pool(name="ps_e", bufs=2, space="PSUM"))

    # ---------------- Phase 0: constants + mids = q @ W.T ----------------
    # W/q DMAs go first on the sync (HWDGE) queue, ahead of the k streams;
    # everything mids-related is latency-critical for the first dot products.
    w_sb = setup.tile([P, 2, D], F32, tag="w")
    nc.sync.dma_start(out=w_sb[:], in_=W.ap().rearrange("(dc p) e -> p dc e", p=P))
    q_sb = setup.tile([BL, D], F32, tag="q")
    nc.sync.dma_start(out=q_sb[:], in_=q.ap())

    ident = consts.tile([P, P], F32)
    make_identity(nc, ident)
    ones1 = consts.tile([1, P], F32)
    nc.vector.memset(ones1[:], 1.0)

    bias_col = consts.tile([P, 1], F32)
    nc.gpsimd.dma_start(out=bias_col[:], in_=_broadcast_row(bias.ap(), P))

    # ones block-diagonal [64, 2]: blk[p, g] = 1 iff p//32 == g
    blk = consts.tile([64, 2], F32)
    nc.gpsimd.memset(blk[:], 1.0)
    nc.gpsimd.affine_select(   # keep where p - 32g >= 0
        out=blk[:], in_=blk[:], compare_op=ALU.is_ge, fill=0.0,
        base=0, pattern=[[-32, 2]], channel_multiplier=1,
    )
    nc.gpsimd.affine_select(   # keep where 31 - p + 32g >= 0  (i.e. p - 32g <= 31)
        out=blk[:], in_=blk[:], compare_op=ALU.is_ge, fill=0.0,
        base=31, pattern=[[32, 2]], channel_multiplier=-1,
    )
    # selector [2, 64]: sel[g, x] = 1 iff x//32 == g
    sel = consts.tile([2, 64], F32)
    nc.gpsimd.memset(sel[:], 0.0)
    nc.gpsimd.affine_select(   # iota = g - (x//32); equal -> fill... keep where != 0
        out=sel.rearrange("p (g x) -> p g x", g=2),
        in_=sel.rearrange("p (g x) -> p g x", g=2),
        compare_op=ALU.not_equal, fill=1.0,
        base=0, pattern=[[-1, 2], [0, 32]], channel_multiplier=1,
    )

    # W^T chunks: wt[p=e_local, ec, dc, d_local]
    wt = setup.tile([P, 2, 2, P], F32, tag="wt")
    for dc in range(2):
        for ec in range(2):
            pst = ps_misc.tile([P, P], F32, tag="mix")
            nc.tensor.transpose(pst[:], w_sb[:, dc, ec * P:(ec + 1) * P], ident[:])
            nc.vector.tensor_copy(wt[:, ec, dc, :], pst[:])
    # q^T chunks: qt[p=e_local, ec, b]
    qt = setup.tile([P, 2, BL], F32, tag="qt")
    for ec in range(2):
        pst = ps_misc.tile([P, BL], F32, tag="mix")
        nc.tensor.transpose(pst[:], q_sb[:, ec * P:(ec + 1) * P], ident[0:BL, 0:BL])
        nc.vector.tensor_copy(qt[:, ec, :], pst[:])
    # midsT[d, b] = sum_e W[d, e] qT[e, b]  (accumulate over 2 e-chunks)
    midsT = setup.tile([P, 2, BL], F32, tag="midsT")
    for dc in range(2):
        psm = ps_misc.tile([P, BL], F32, tag="mix")
        for ec in range(2):
            nc.tensor.matmul(
                psm[:], lhsT=wt[:, ec, dc, :], rhs=qt[:, ec, :],
                start=(ec == 0), stop=(ec == 1),
            )
        nc.vector.tensor_copy(midsT[:, dc, :], psm[:])
    # mids rows [b, d]
    mids = setup.tile([BL, D], F32, tag="mids")
    for dc in range(2):
        psr = ps_misc.tile([BL, P], F32, tag="mix")
        nc.tensor.transpose(psr[:], midsT[:, dc, :], ident[:])
        nc.vector.tensor_copy(mids[:, dc * P:(dc + 1) * P], psr[:])
    # fold the 8 mids rows onto partition 0 (engines need 32-aligned partition
    # bases; partition 0 lets the ones-matmul below read any row as rhs)
    mids_flat = setup.tile([1, BL, D], F32, tag="mids_flat")
    nc.gpsimd.dma_start(out=mids_flat[:], in_=mids[:])

    # ---------------- Phase 1: main loop + epilogue per batch-pair ----------------
    ps_mb = ctx.enter_context(tc.tile_pool(name="ps_mb", bufs=2, space="PSUM"))
    for g in range(BL // 2):                 # 4 pairs
        scores = scpool.tile([P, 64], F32)   # col = b_local*32 + h*16 + tt
        # broadcast this pair's mids rows to all 128 partitions via a K=1
        # ones-matmul (PSUM result is read directly as the STT/TT in1)
        mb_ps = ps_mb.tile([P, 2, D], F32)
        for b_local in range(2):
            b = g * 2 + b_local
            nc.tensor.matmul(
                mb_ps[:, b_local, :], lhsT=ones1[:],
                rhs=mids_flat[:, b, :], start=True, stop=True,
            )
        for b_local in range(2):
            b = g * 2 + b_local
            mb = mb_ps[:, b_local, :]
            for h in range(2):
                kt = kpool.tile([P, TT, D], F32)
                nc.sync.dma_start(
                    out=kt[:],
                    in_=k.ap()[b, h * 2048:(h + 1) * 2048, :].rearrange(
                        "(p tt) d -> p tt d", p=P
                    ),
                )
                c0 = b_local * 32 + h * 16
                # TT chunks first so ACT's reduces start as early as possible
                tt0 = N_TTR
                for clen in CHUNKS:
                    tmp = scratch.tile([P, max(CHUNKS), D], F32, tag="mulchunk")
                    nc.vector.tensor_tensor(
                        out=tmp[:, 0:clen, :],
                        in0=kt[:, tt0:tt0 + clen, :],
                        in1=mb.unsqueeze(1).broadcast_to([P, clen, D]),
                        op=ALU.mult,
                    )
                    for i in range(clen):
                        asc = scratch.tile([P, D], F32, tag="actred")
                        nc.scalar.activation(
                            out=asc[:], in_=tmp[:, i, :], func=AF.Copy,
                            accum_out=scores[:, c0 + tt0 + i:c0 + tt0 + i + 1],
                        )
                    tt0 += clen
                for tt in range(N_TTR):
                    sc = scratch.tile([P, D], F32, tag="ttr")
                    nc.vector.scalar_tensor_tensor(
                        out=sc[:], in0=kt[:, tt, :], scalar=0.0, in1=mb,
                        op0=ALU.bypass, op1=ALU.mult,
                        accum_out=scores[:, c0 + tt:c0 + tt + 1],
                    )

        # ---- epilogue for this pair of batches ----
        th = epool.tile([P, 64], F32, tag="th")
        nc.scalar.activation(out=th[:], in_=scores[:], func=AF.Tanh,
                             bias=bias_col[:], scale=1.0)
        ex = epool.tile([P, 64], F32, tag="ex")
        nc.scalar.activation(out=ex[:], in_=th[:], func=AF.Exp)
        pse = ps_e.tile([64, P], F32)
        nc.tensor.transpose(pse[:], ex[:], ident[:])

        mt = epool.tile([64, P], F32, tag="mt")
        nc.gpsimd.dma_start(
            out=mt[:],
            in_=mp.ap()[g * 2:(g + 1) * 2].rearrange("b c p -> (b c) p"),
        )
        ee = epool.tile([64, P], F32, tag="ee")
        rs = epool.tile([64, 1], F32, tag="rs")
        nc.vector.scalar_tensor_tensor(
            out=ee[:], in0=pse[:], scalar=0.0, in1=mt[:],
            op0=ALU.bypass, op1=ALU.mult, accum_out=rs[:],
        )
        pss = ps_misc.tile([2, 1], F32, tag="mix")
        nc.tensor.matmul(pss[:], lhsT=blk[:], rhs=rs[:], start=True, stop=True)
        rc = epool.tile([2, 1], F32, tag="rc")
        nc.vector.reciprocal(rc[:], pss[:])
        psr2 = ps_misc.tile([64, 1], F32, tag="mix")
        nc.tensor.matmul(psr2[:], lhsT=sel[:], rhs=rc[:], start=True, stop=True)
        rcol = epool.tile([64, 1], F32, tag="rcol")
        nc.vector.tensor_copy(rcol[:], psr2[:])
        attn = epool.tile([64, P], F32, tag="attn")
        nc.scalar.activation(out=attn[:], in_=ee[:], func=AF.Copy, scale=rcol[:])
        nc.gpsimd.dma_start(
            out=out.ap()[g * 2:(g + 1) * 2].rearrange("b c p -> (b c) p"),
            in_=attn[:],
        )


def _install_ntff_hook_shim():
    """Provide antenv.axon_hooks via ctypes into libaxon_pjrt.so (the agent
    image's antenv stub lacks it), enabling NTFF capture under trace=True."""
    import sys
    import types
    import ctypes
    import contextlib

    if "antenv.axon_hooks" in sys.modules:
        return
    so = "/opt/axon/libaxon_pjrt.so"
    if not os.path.exists(so):
        return
    lib = ctypes.CDLL(so)
    if not hasattr(lib, "axon_start_nrt_profile"):
        return
    lib.axon_start_nrt_profile.argtypes = [
        ctypes.POINTER(ctypes.c_int64), ctypes.c_size_t,
    ]
    lib.axon_start_nrt_profile.restype = ctypes.c_int64
    lib.axon_stop_nrt_profile.argtypes = [ctypes.c_char_p]
    lib.axon_stop_nrt_profile.restype = ctypes.c_int64

    @contextlib.contextmanager
    def _hook(output_dir, device_ids):
        import jax

        jax.devices()
        if device_ids:
            ids = (ctypes.c_int64 * len(device_ids))(*device_ids)
            rc = lib.axon_start_nrt_profile(ids, len(device_ids))
        else:
            rc = lib.axon_start_nrt_profile(None, 0)
        if rc != 0:
            raise RuntimeError(f"axon_start_nrt_profile rc={rc}")
        try:
            yield
        finally:
            n = lib.axon_stop_nrt_profile(str(output_dir).encode())
            print(f"profile: {n} file(s) written to {output_dir}", file=sys.stderr)

    mod = types.ModuleType("antenv.axon_hooks")
    mod.get_axon_ntff_profile_hook = lambda: _hook
    mod.set_axon_ntff_profile_hook = lambda h: None
    import antenv

    sys.modules["antenv.axon_hooks"] = mod
    antenv.axon_hooks = mod


_CACHE = {}


def _get_nc():
    if "nc" not in _CACHE:
        from contextlib import ExitStack

        nc = bacc.Bacc("TRN2", debug=False)
        ins = {
            "q": nc.dram_tensor("q", [BL, D], F32, kind="ExternalInput"),
            "k": nc.dram_tensor("k", [BL, T, D], F32, kind="ExternalInput"),
            "mp": nc.dram_tensor("mp", [BL, 32, P], F32, kind="ExternalInput"),
            "W": nc.dram_tensor("W", [D, D], F32, kind="ExternalInput"),
            "bias": nc.dram_tensor("bias", [1, 1], F32, kind="ExternalInput"),
        }
        outs = {"out": nc.dram_tensor("out", [BL, 32, P], F32, kind="ExternalOutput")}
        with tile.TileContext(nc) as tc:
            with ExitStack() as ctx:
                _build_kernel(ctx, tc, outs, ins)
        nc.compile()
        _CACHE["nc"] = nc
    return _CACHE["nc"]


def kernel(q, k, m, W, bias):
    global LAST_RESULTS
    q = np.ascontiguousarray(q, dtype=np.float32)
    k = np.ascontiguousarray(k, dtype=np.float32)
    m = np.ascontiguousarray(m, dtype=np.float32)
    W = np.ascontiguousarray(W, dtype=np.float32)
    bias = np.ascontiguousarray(bias, dtype=np.float32).reshape(1, 1)

    # host-side input marshalling: permute m to the kernel's score layout
    # mp[b, h*16+tt, p] = m[b, h*2048 + p*16 + tt]
    mp = np.ascontiguousarray(
        m.reshape(B, H, P, TT).transpose(0, 1, 3, 2).reshape(B, H * TT, P)
    )

    trace = bool(int(os.environ.get("KERNEL_TRACE", "0")))
    if trace:
        _install_ntff_hook_shim()
    nc = _get_nc()
    in_maps = [
        {
            "q": q[i * BL:(i + 1) * BL],
            "k": k[i * BL:(i + 1) * BL],
            "mp": mp[i * BL:(i + 1) * BL],
            "W": W,
            "bias": bias,
        }
        for i in range(NCORES)
    ]
    res = run_bass_kernel_spmd(
        nc,
        in_maps,
        core_ids=list(range(NCORES)),
        trace=trace,
    )
    LAST_RESULTS = res

    full = np.concatenate([res.results[i]["out"] for i in range(NCORES)], axis=0)
    # inverse permutation back to natural [B, T]
    out = np.ascontiguousarray(
        full.reshape(B, H, TT, P).transpose(0, 1, 3, 2).reshape(B, T)
    )
    return out
